# revision 42
# baseline (speedup 1.0000x reference)
"""Octahedral SHT on 8 NeuronCores (Bass/Tile) — v2.

v1 -> v2: per-ring PSUM accumulation (chunks of one ring accumulate on-chip
before the DRAM bounce) and 4-way m-packing in phase 2.

Sharding: 204 north DFT chunks -> 8 cores x 27 slots, organized as 12
ring-groups per core with the uniform size pattern [4,3,3,3,3,2,2,2,2,1,1,1]
(27 slots). Ring classes fit exactly: 4-groups take the 4 four-chunk rings +
4 two-chunk rings (padded), 3-groups the 32 three-chunk rings, 2-groups
28 two-chunk + 4 one-chunk rings, 1-groups 24 one-chunk rings. Each slot
also carries the mirrored south ring's chunk (identical DFT matrix E since
nlon is north/south symmetric), halving E traffic and PE weight loads.

Phase 1 (per group): psum[m, 512] = [re_n|re_s | im_n|im_s] accumulated
over the group's chunks (2 matmuls per slot, start on first chunk / stop on
last). Evacuate fp32->fp16 with 4 copies reordering to ring-major rows,
DMA 2 rows to gdram [24 rows, 128 m, 256] (row = ring: [re|im] per m).
Phase 2 (per 16-m group, per quad qi<4): the quad covers m = 16g+qi+4b for
bands b=0..3; gsb [96, 1024] row 4r+b <- gdram[r, m-block 16g+4b+qi] (the
16 m-blocks per ring are contiguous, so each group load is one straight
[24, 8KB] DMA). 2 matmuls per quad, K=96: lhsT = G_re/G_im [96, 128 bev],
rhs = block-diag pw [96, 4L] (row 4r+b -> pw[ring r, m(b)] at cols
[bL,(b+1)L)). Only l >= 16*(m//16) is computed (coeffs with l < m are
structurally zero). Output fp16 [128 bev, 18432]; host sums the 8 partials
and unpacks the triangle.
"""
import numpy as np

NLAT, LMAX, MMAX = 192, 128, 128
B, V = 2, 64
BF = B * V
NCORES = 8
CHUNK = 128
GSIZES = [4, 3, 3, 3, 3, 2, 2, 2, 2, 1, 1, 1]
NG = len(GSIZES)                  # 12 ring-groups per core
NSLOT = sum(GSIZES)               # 27
NROWS = 2 * NG                    # 24 G rows per core
GOFF = np.concatenate([[0], np.cumsum(GSIZES)]).astype(np.int64)
MAX_NLON = 400
NPTS = 40320


def LB(m):
    return 16 * (m // 16)


def LLEN(m):
    return LMAX - LB(m)


# quad q = 4*grp + qi -> m's { 16*grp + qi + 4*b : b in 0..3 }, L = 128-16*grp
QUAD_OFF = np.zeros(32, np.int64)
_o = 0
for _q in range(32):
    QUAD_OFF[_q] = _o
    _o += 8 * LLEN(16 * (_q // 4))
OUTW = int(_o)                    # 18432

QD_OFF = np.zeros(32, np.int64)
_o = 0
for _q in range(32):
    QD_OFF[_q] = _o
    _o += 4 * LLEN(16 * (_q // 4))
PWDW = int(_o)                    # 9216


def _octa_nlon():
    half = NLAT // 2
    north = np.array([4 * (i + 1) + 16 for i in range(half)], dtype=np.int64)
    return np.concatenate([north, north[::-1]])


def _assign_groups():
    """Per-core list of NG north rings (group g -> one ring, padded to
    GSIZES[g] chunk slots)."""
    nlon = _octa_nlon()
    nch = np.ceil(nlon[:96] / CHUNK).astype(int)
    cls = {c: sorted(np.where(nch == c)[0].tolist()) for c in (1, 2, 3, 4)}
    assert [len(cls[c]) for c in (1, 2, 3, 4)] == [28, 32, 32, 4]
    c1, c2, c3, c4 = cls[1][:], cls[2][:], cls[3][:], cls[4][:]
    cores = []
    for c in range(NCORES):
        g4 = c4.pop() if c < 4 else c2.pop()
        g3s = [c3.pop() for _ in range(4)]
        g2s = [c2.pop() for _ in range(4)] if c < 4 else \
              [c2.pop() for _ in range(3)] + [c1.pop()]
        g1s = [c1.pop() for _ in range(3)]
        cores.append([g4] + g3s + g2s + g1s)
    assert not c1 and not c2 and not c3 and not c4
    return cores, nlon


def _build_core(rings, nlon, offs, x, E_re, E_im, PwT):
    xe = np.zeros((CHUNK, NSLOT, 2, BF), np.float16)
    ee = np.zeros((CHUNK, NSLOT, 2 * MMAX), np.float16)
    pwc = np.zeros((NROWS, MMAX, LMAX), np.float32)
    for g in range(NG):
        r = rings[g]
        rs = NLAT - 1 - r
        nl = int(nlon[r])
        for t in range(GSIZES[g]):
            j0 = t * CHUNK
            if j0 >= nl:
                continue
            s = int(GOFF[g]) + t
            jlen = min(CHUNK, nl - j0)
            xe[:jlen, s, 0, :] = x[:, offs[r] + j0: offs[r] + j0 + jlen].T
            xe[:jlen, s, 1, :] = x[:, offs[rs] + j0: offs[rs] + j0 + jlen].T
            elen = min(CHUNK, MAX_NLON - j0)
            ee[:elen, s, 0:MMAX] = E_re[r, j0:j0 + elen, :]
            ee[:elen, s, MMAX:] = E_im[r, j0:j0 + elen, :]
        pwc[2 * g] = PwT[:, r, :]          # [m, l]
        pwc[2 * g + 1] = PwT[:, rs, :]
    # pwd row 4*r + b pairs with gsb row (ring r, band b)
    pwd = np.zeros((4 * NROWS, PWDW), np.float16)
    for q in range(32):
        grp, qi = divmod(q, 4)
        L = LLEN(16 * grp)
        lb = 16 * grp
        o = int(QD_OFF[q])
        for b in range(4):
            m = 16 * grp + qi + 4 * b
            pwd[4 * np.arange(NROWS) + b, o + b * L:o + (b + 1) * L] = \
                pwc[:, m, lb:]
    return (np.ascontiguousarray(xe.reshape(CHUNK, NSLOT * 256)),
            np.ascontiguousarray(ee.reshape(CHUNK, NSLOT * 256)),
            pwd)


def _build_bass():
    import concourse.mybir as mybir
    from concourse import bacc, tile

    dt = mybir.dt
    nc = bacc.Bacc()

    xe_d = nc.dram_tensor("xe", [CHUNK, NSLOT * 256], dt.float16,
                          kind="ExternalInput")
    ee_d = nc.dram_tensor("ee", [CHUNK, NSLOT * 256], dt.float16,
                          kind="ExternalInput")
    pwd_d = nc.dram_tensor("pwd", [4 * NROWS, PWDW], dt.float16,
                           kind="ExternalInput")
    outp_d = nc.dram_tensor("outp", [128, OUTW], dt.float16,
                            kind="ExternalOutput")
    gdram = nc.dram_tensor("gdram", [NROWS, MMAX * 256], dt.float16)

    with tile.TileContext(nc) as tc:
        with (
            tc.tile_pool(name="inp", bufs=1) as in_pool,
            tc.tile_pool(name="gsl", bufs=6) as gsl_pool,
            tc.tile_pool(name="gsb", bufs=8) as gsb_pool,
            tc.tile_pool(name="osb", bufs=3) as osb_pool,
            tc.tile_pool(name="ps1a", bufs=2, space="PSUM") as ps1a,
            tc.tile_pool(name="ps1b", bufs=2, space="PSUM") as ps1b,
            tc.tile_pool(name="ps2", bufs=4, space="PSUM") as ps2,
        ):
            xe = in_pool.tile([CHUNK, NSLOT * 256], dt.float16, tag="xe")
            ee = in_pool.tile([CHUNK, NSLOT * 256], dt.float16, tag="ee")
            pwd = in_pool.tile([4 * NROWS, PWDW], dt.float16, tag="pwd")

            # loads: xe on sync, ee on gpsimd (scalar stays copy-only in
            # phase 1); pwd late on gpsimd (phase-2 input). First chunk is
            # small so group-0 matmuls start early.
            GBL = [0, 4, 9, 15, 21, 27]
            for g in range(5):
                c0, c1 = GBL[g] * 256, GBL[g + 1] * 256
                xq = nc.sync if g % 2 == 0 else nc.scalar
                xq.dma_start(out=xe[:, c0:c1], in_=xe_d[:, c0:c1])
                nc.gpsimd.dma_start(out=ee[:, c0:c1], in_=ee_d[:, c0:c1])
            nc.gpsimd.dma_start(out=pwd[:], in_=pwd_d[:])

            cp_engines = [nc.scalar, nc.vector]

            def cp(idx, out, in_):
                e = cp_engines[idx % 2]
                if e is nc.scalar:
                    e.copy(out, in_)
                else:
                    e.tensor_copy(out, in_)

            # ---- phase 1: 12 ring-groups, psum accumulation over chunks ----
            ci = 0
            gsl = None
            for g in range(NG):
                sz = GSIZES[g]
                # separate banks for the re / im accumulation chains (one
                # psum zero-region cannot host two pending groups)
                gre = ps1a.tile([MMAX, 512], dt.float32, tag="gre")
                gim = ps1b.tile([MMAX, 512], dt.float32, tag="gim")
                for t in range(sz):
                    s = int(GOFF[g]) + t
                    rhs = xe[:, s * 256:(s + 1) * 256]
                    st, sp = (t == 0), (t == sz - 1)
                    nc.tensor.matmul(gre[:, 0:256],
                                     ee[:, s * 256:s * 256 + 128],
                                     rhs, start=st, stop=sp)
                    nc.tensor.matmul(gim[:, 0:256],
                                     ee[:, s * 256 + 128:(s + 1) * 256],
                                     rhs, start=st, stop=sp)
                if g % 2 == 0:
                    gsl = gsl_pool.tile([MMAX, 1024], dt.float16, tag="gsl")
                go = 512 * (g % 2)
                # [re_n|re_s] + [im_n|im_s] -> ring-major [re_n|im_n|re_s|im_s]
                cp(ci + 0, gsl[:, go + 0:go + 128], gre[:, 0:128])
                cp(ci + 1, gsl[:, go + 128:go + 256], gim[:, 0:128])
                cp(ci + 2, gsl[:, go + 256:go + 384], gre[:, 128:256])
                cp(ci + 3, gsl[:, go + 384:go + 512], gim[:, 128:256])
                ci += 4
                if g % 2 == 1:
                    # one write per group pair (4 ring rows): fewer issue
                    # slots on the sync queue shorten the phase barrier
                    dst = gdram[2 * g - 2:2 * g + 2].rearrange(
                        "k (m c) -> m k c", m=MMAX)
                    nc.sync.dma_start(out=dst, in_=gsl[:])

            # ---- phase 2: 8 m-groups x 4 quads, one osb + out DMA per grp ----
            # prefetch all gsb blocks upfront (split over two queues).
            # gsb row 4*r + b <- gdram[r, m-block mg+4b+qi]: the 16 m-blocks
            # per ring are contiguous, so each is a straight [24, 8KB] copy
            gsbs = []
            for grp in range(8):
                gsb = gsb_pool.tile([4 * NROWS, 1024], dt.float16, tag="gsb")
                gq = nc.scalar if grp % 2 == 0 else nc.gpsimd
                gq.dma_start(out=gsb[:],
                             in_=gdram[:, grp * 16 * 256:(grp + 1) * 16 * 256])
                gsbs.append(gsb)
            for grp in range(8):
                L = 128 - 16 * grp
                gsb = gsbs[grp]
                gbase = int(QUAD_OFF[4 * grp])
                osb = osb_pool.tile([128, 32 * L], dt.float16, tag="osb")
                per_bank = max(1, 512 // (8 * L))
                po = None
                if L > 64:
                    # quad re / im each need their own psum bank (4L > 256)
                    for qi in range(4):
                        q = 4 * grp + qi
                        do = int(QD_OFF[q])
                        prhs = pwd[:, do:do + 4 * L]
                        qo = int(QUAD_OFF[q]) - gbase
                        for h in range(2):   # 0: re, 1: im
                            po = ps2.tile([128, 512], dt.float32, tag="po")
                            nc.tensor.matmul(
                                po[:, 0:4 * L],
                                gsb[:, qi * 256 + h * 128:qi * 256 + h * 128 + 128],
                                prhs, start=True, stop=True)
                            cp(ci, osb[:, qo + h * 4 * L:qo + (h + 1) * 4 * L],
                               po[:, 0:4 * L])
                            ci += 1
                else:
                    # whole quads (re+im = 8L <= 512) pack into banks
                    wb = per_bank * 8 * L
                    for qi in range(4):
                        q = 4 * grp + qi
                        do = int(QD_OFF[q])
                        prhs = pwd[:, do:do + 4 * L]
                        bq = qi % per_bank
                        if bq == 0:
                            po = ps2.tile([128, 512], dt.float32, tag="po")
                        c0 = bq * 8 * L
                        nc.tensor.matmul(po[:, c0:c0 + 4 * L],
                                         gsb[:, qi * 256:qi * 256 + 128],
                                         prhs, start=True, stop=True)
                        nc.tensor.matmul(po[:, c0 + 4 * L:c0 + 8 * L],
                                         gsb[:, qi * 256 + 128:(qi + 1) * 256],
                                         prhs, start=True, stop=True)
                        if bq == per_bank - 1:
                            qo = int(QUAD_OFF[q - per_bank + 1]) - gbase
                            cp(ci, osb[:, qo:qo + wb], po[:, 0:wb])
                            ci += 1
                we = (nc.gpsimd, nc.sync, nc.scalar)[grp % 3]
                we.dma_start(out=outp_d[:, gbase:gbase + 32 * L], in_=osb[:])

    nc.compile()
    return nc


_CACHE = {}


def _get_compiled():
    if "nc" not in _CACHE:
        _CACHE["nc"] = _build_bass()
    return _CACHE["nc"]


def kernel(data, Pw, E_re, E_im, pad_idx):
    from concourse import bass_utils

    data = np.asarray(data)
    Pw = np.asarray(Pw, dtype=np.float32)
    E_re = np.asarray(E_re, dtype=np.float32)
    E_im = np.asarray(E_im, dtype=np.float32)

    cores, nlon = _assign_groups()
    offs = np.concatenate([[0], np.cumsum(nlon)[:-1]])
    x = np.ascontiguousarray(
        np.transpose(data, (0, 1, 3, 2)).reshape(BF, NPTS).astype(np.float32))
    PwT = np.ascontiguousarray(np.transpose(Pw, (1, 2, 0)))  # [m, n, l]

    in_maps = []
    for c in range(NCORES):
        xe, ee, pwd = _build_core(cores[c], nlon, offs, x, E_re, E_im, PwT)
        in_maps.append({"xe": xe, "ee": ee, "pwd": pwd})

    nc = _get_compiled()
    res = bass_utils.run_bass_kernel_spmd(nc, in_maps, list(range(NCORES)))
    _CACHE["last_results"] = res

    total = np.zeros((BF, OUTW), np.float64)
    for r in res.results:
        total += r["outp"].astype(np.float64)
    total = total.astype(np.float32)

    cc = np.zeros((LMAX, MMAX, BF), np.complex64)
    for q in range(32):
        grp, qi = divmod(q, 4)
        L = LLEN(16 * grp)
        lb = 16 * grp
        o = int(QUAD_OFF[q])
        for b in range(4):
            m = 16 * grp + qi + 4 * b
            re = total[:, o + b * L:o + (b + 1) * L]
            im = total[:, o + 4 * L + b * L:o + 4 * L + (b + 1) * L]
            cc[lb:, m, :] = (re + 1j * im).T
    cc = cc.reshape(LMAX, MMAX, B, V)
    out = np.transpose(cc, (2, 0, 1, 3))[:, None]
    return out.astype(np.complex64)


# revision 44
# speedup vs baseline: 1.0256x; 1.0256x over previous
"""Octahedral SHT on 8 NeuronCores (Bass/Tile) — v2.

v1 -> v2: per-ring PSUM accumulation (chunks of one ring accumulate on-chip
before the DRAM bounce) and 4-way m-packing in phase 2.

Sharding: 204 north DFT chunks -> 8 cores x 27 slots, organized as 12
ring-groups per core with the uniform size pattern [4,3,3,3,3,2,2,2,2,1,1,1]
(27 slots). Ring classes fit exactly: 4-groups take the 4 four-chunk rings +
4 two-chunk rings (padded), 3-groups the 32 three-chunk rings, 2-groups
28 two-chunk + 4 one-chunk rings, 1-groups 24 one-chunk rings. Each slot
also carries the mirrored south ring's chunk (identical DFT matrix E since
nlon is north/south symmetric), halving E traffic and PE weight loads.

Phase 1 (per group): psum[m, 512] = [re_n|re_s | im_n|im_s] accumulated
over the group's chunks (2 matmuls per slot, start on first chunk / stop on
last). Evacuate fp32->fp16 with 4 copies reordering to ring-major rows,
DMA 2 rows to gdram [24 rows, 128 m, 256] (row = ring: [re|im] per m).
Phase 2 (per 16-m group, per quad qi<4): the quad covers m = 16g+qi+4b for
bands b=0..3; gsb [96, 1024] row 4r+b <- gdram[r, m-block 16g+4b+qi] (the
16 m-blocks per ring are contiguous, so each group load is one straight
[24, 8KB] DMA). 2 matmuls per quad, K=96: lhsT = G_re/G_im [96, 128 bev],
rhs = block-diag pw [96, 4L] (row 4r+b -> pw[ring r, m(b)] at cols
[bL,(b+1)L)). Only l >= 16*(m//16) is computed (coeffs with l < m are
structurally zero). Output fp16 [128 bev, 18432]; host sums the 8 partials
and unpacks the triangle.
"""
import numpy as np

NLAT, LMAX, MMAX = 192, 128, 128
B, V = 2, 64
BF = B * V
NCORES = 8
CHUNK = 128
GSIZES = [4, 3, 3, 3, 3, 2, 2, 2, 2, 1, 1, 1]
NG = len(GSIZES)                  # 12 ring-groups per core
NSLOT = sum(GSIZES)               # 27
NROWS = 2 * NG                    # 24 G rows per core
GOFF = np.concatenate([[0], np.cumsum(GSIZES)]).astype(np.int64)
MAX_NLON = 400
NPTS = 40320


def LB(m):
    return 16 * (m // 16)


def LLEN(m):
    return LMAX - LB(m)


# quad q = 4*grp + qi -> m's { 16*grp + qi + 4*b : b in 0..3 }, L = 128-16*grp
QUAD_OFF = np.zeros(32, np.int64)
_o = 0
for _q in range(32):
    QUAD_OFF[_q] = _o
    _o += 8 * LLEN(16 * (_q // 4))
OUTW = int(_o)                    # 18432

QD_OFF = np.zeros(32, np.int64)
_o = 0
for _q in range(32):
    QD_OFF[_q] = _o
    _o += 4 * LLEN(16 * (_q // 4))
PWDW = int(_o)                    # 9216


def _octa_nlon():
    half = NLAT // 2
    north = np.array([4 * (i + 1) + 16 for i in range(half)], dtype=np.int64)
    return np.concatenate([north, north[::-1]])


def _assign_groups():
    """Per-core list of NG north rings (group g -> one ring, padded to
    GSIZES[g] chunk slots)."""
    nlon = _octa_nlon()
    nch = np.ceil(nlon[:96] / CHUNK).astype(int)
    cls = {c: sorted(np.where(nch == c)[0].tolist()) for c in (1, 2, 3, 4)}
    assert [len(cls[c]) for c in (1, 2, 3, 4)] == [28, 32, 32, 4]
    c1, c2, c3, c4 = cls[1][:], cls[2][:], cls[3][:], cls[4][:]
    cores = []
    for c in range(NCORES):
        g4 = c4.pop() if c < 4 else c2.pop()
        g3s = [c3.pop() for _ in range(4)]
        g2s = [c2.pop() for _ in range(4)] if c < 4 else \
              [c2.pop() for _ in range(3)] + [c1.pop()]
        g1s = [c1.pop() for _ in range(3)]
        cores.append([g4] + g3s + g2s + g1s)
    assert not c1 and not c2 and not c3 and not c4
    return cores, nlon


def _build_core(rings, nlon, offs, x, E_re, E_im, PwT):
    xe = np.zeros((CHUNK, NSLOT, 2, BF), np.float16)
    ee = np.zeros((CHUNK, NSLOT, 2 * MMAX), np.float16)
    pwc = np.zeros((NROWS, MMAX, LMAX), np.float32)
    for g in range(NG):
        r = rings[g]
        rs = NLAT - 1 - r
        nl = int(nlon[r])
        for t in range(GSIZES[g]):
            j0 = t * CHUNK
            if j0 >= nl:
                continue
            s = int(GOFF[g]) + t
            jlen = min(CHUNK, nl - j0)
            xe[:jlen, s, 0, :] = x[:, offs[r] + j0: offs[r] + j0 + jlen].T
            xe[:jlen, s, 1, :] = x[:, offs[rs] + j0: offs[rs] + j0 + jlen].T
            elen = min(CHUNK, MAX_NLON - j0)
            ee[:elen, s, 0:MMAX] = E_re[r, j0:j0 + elen, :]
            ee[:elen, s, MMAX:] = E_im[r, j0:j0 + elen, :]
        pwc[2 * g] = PwT[:, r, :]          # [m, l]
        pwc[2 * g + 1] = PwT[:, rs, :]
    # pwd row 4*r + b pairs with gsb row (ring r, band b)
    pwd = np.zeros((4 * NROWS, PWDW), np.float16)
    for q in range(32):
        grp, qi = divmod(q, 4)
        L = LLEN(16 * grp)
        lb = 16 * grp
        o = int(QD_OFF[q])
        for b in range(4):
            m = 16 * grp + qi + 4 * b
            pwd[4 * np.arange(NROWS) + b, o + b * L:o + (b + 1) * L] = \
                pwc[:, m, lb:]
    return (np.ascontiguousarray(xe.reshape(CHUNK, NSLOT * 256)),
            np.ascontiguousarray(ee.reshape(CHUNK, NSLOT * 256)),
            pwd)


def _build_bass():
    import concourse.mybir as mybir
    from concourse import bacc, tile

    dt = mybir.dt
    nc = bacc.Bacc()

    xe_d = nc.dram_tensor("xe", [CHUNK, NSLOT * 256], dt.float16,
                          kind="ExternalInput")
    ee_d = nc.dram_tensor("ee", [CHUNK, NSLOT * 256], dt.float16,
                          kind="ExternalInput")
    pwd_d = nc.dram_tensor("pwd", [4 * NROWS, PWDW], dt.float16,
                           kind="ExternalInput")
    outp_d = nc.dram_tensor("outp", [128, OUTW], dt.float16,
                            kind="ExternalOutput")
    gdram = nc.dram_tensor("gdram", [NROWS, MMAX * 256], dt.float16)

    with tile.TileContext(nc) as tc:
        with (
            tc.tile_pool(name="inp", bufs=1) as in_pool,
            tc.tile_pool(name="gsl", bufs=6) as gsl_pool,
            tc.tile_pool(name="gsb", bufs=8) as gsb_pool,
            tc.tile_pool(name="osb", bufs=3) as osb_pool,
            tc.tile_pool(name="ps1a", bufs=2, space="PSUM") as ps1a,
            tc.tile_pool(name="ps1b", bufs=2, space="PSUM") as ps1b,
            tc.tile_pool(name="ps2", bufs=4, space="PSUM") as ps2,
        ):
            xe = in_pool.tile([CHUNK, NSLOT * 256], dt.float16, tag="xe")
            ee = in_pool.tile([CHUNK, NSLOT * 256], dt.float16, tag="ee")
            pwd = in_pool.tile([4 * NROWS, PWDW], dt.float16, tag="pwd")

            # loads: xe on sync, ee on gpsimd (scalar stays copy-only in
            # phase 1); pwd late on gpsimd (phase-2 input). First chunk is
            # small so group-0 matmuls start early.
            GBL = [0, 4, 9, 15, 21, 27]
            for g in range(5):
                c0, c1 = GBL[g] * 256, GBL[g + 1] * 256
                xq = nc.sync if g % 2 == 0 else nc.scalar
                xq.dma_start(out=xe[:, c0:c1], in_=xe_d[:, c0:c1])
                nc.gpsimd.dma_start(out=ee[:, c0:c1], in_=ee_d[:, c0:c1])
            nc.gpsimd.dma_start(out=pwd[:], in_=pwd_d[:])

            cp_engines = [nc.scalar, nc.vector]

            def cp(idx, out, in_):
                e = cp_engines[idx % 2]
                if e is nc.scalar:
                    e.copy(out, in_)
                else:
                    e.tensor_copy(out, in_)

            # ---- phase 1: 12 ring-groups, psum accumulation over chunks ----
            ci = 0
            gsl = None
            for g in range(NG):
                sz = GSIZES[g]
                # separate banks for the re / im accumulation chains (one
                # psum zero-region cannot host two pending groups)
                gre = ps1a.tile([MMAX, 512], dt.float32, tag="gre")
                gim = ps1b.tile([MMAX, 512], dt.float32, tag="gim")
                for t in range(sz):
                    s = int(GOFF[g]) + t
                    rhs = xe[:, s * 256:(s + 1) * 256]
                    st, sp = (t == 0), (t == sz - 1)
                    nc.tensor.matmul(gre[:, 0:256],
                                     ee[:, s * 256:s * 256 + 128],
                                     rhs, start=st, stop=sp)
                    nc.tensor.matmul(gim[:, 0:256],
                                     ee[:, s * 256 + 128:(s + 1) * 256],
                                     rhs, start=st, stop=sp)
                if g % 2 == 0:
                    gsl = gsl_pool.tile([MMAX, 1024], dt.float16, tag="gsl")
                go = 512 * (g % 2)
                # [re_n|re_s] + [im_n|im_s] -> ring-major [re_n|im_n|re_s|im_s]
                cp(ci + 0, gsl[:, go + 0:go + 128], gre[:, 0:128])
                cp(ci + 1, gsl[:, go + 128:go + 256], gim[:, 0:128])
                cp(ci + 2, gsl[:, go + 256:go + 384], gre[:, 128:256])
                cp(ci + 3, gsl[:, go + 384:go + 512], gim[:, 128:256])
                ci += 4
                if g >= NG - 2:
                    # last two groups write singly so the final (barrier-
                    # gating) write is as small and early as possible
                    dst = gdram[2 * g:2 * g + 2].rearrange(
                        "k (m c) -> m k c", m=MMAX)
                    nc.sync.dma_start(out=dst, in_=gsl[:, go:go + 512])
                elif g % 2 == 1:
                    # one write per group pair (4 ring rows): fewer issue
                    # slots on the sync queue shorten the phase barrier
                    dst = gdram[2 * g - 2:2 * g + 2].rearrange(
                        "k (m c) -> m k c", m=MMAX)
                    nc.sync.dma_start(out=dst, in_=gsl[:])

            # ---- phase 2: 8 m-groups x 4 quads, one osb + out DMA per grp ----
            # prefetch all gsb blocks upfront (split over two queues).
            # gsb row 4*r + b <- gdram[r, m-block mg+4b+qi]: the 16 m-blocks
            # per ring are contiguous, so each is a straight [24, 8KB] copy
            gsbs = []
            for grp in range(8):
                gsb = gsb_pool.tile([4 * NROWS, 1024], dt.float16, tag="gsb")
                if grp == 0:
                    # the load gating phase-2 start: two parallel halves
                    nc.gpsimd.dma_start(out=gsb[0:64, :],
                                        in_=gdram[0:16, 0:16 * 256])
                    nc.scalar.dma_start(out=gsb[64:96, :],
                                        in_=gdram[16:24, 0:16 * 256])
                else:
                    gq = nc.scalar if grp % 2 == 0 else nc.gpsimd
                    gq.dma_start(
                        out=gsb[:],
                        in_=gdram[:, grp * 16 * 256:(grp + 1) * 16 * 256])
                gsbs.append(gsb)
            for grp in range(8):
                L = 128 - 16 * grp
                gsb = gsbs[grp]
                gbase = int(QUAD_OFF[4 * grp])
                osb = osb_pool.tile([128, 32 * L], dt.float16, tag="osb")
                per_bank = max(1, 512 // (8 * L))
                po = None
                if L > 64:
                    # quad re / im each need their own psum bank (4L > 256)
                    for qi in range(4):
                        q = 4 * grp + qi
                        do = int(QD_OFF[q])
                        prhs = pwd[:, do:do + 4 * L]
                        qo = int(QUAD_OFF[q]) - gbase
                        for h in range(2):   # 0: re, 1: im
                            po = ps2.tile([128, 512], dt.float32, tag="po")
                            nc.tensor.matmul(
                                po[:, 0:4 * L],
                                gsb[:, qi * 256 + h * 128:qi * 256 + h * 128 + 128],
                                prhs, start=True, stop=True)
                            cp(ci, osb[:, qo + h * 4 * L:qo + (h + 1) * 4 * L],
                               po[:, 0:4 * L])
                            ci += 1
                else:
                    # whole quads (re+im = 8L <= 512) pack into banks
                    wb = per_bank * 8 * L
                    for qi in range(4):
                        q = 4 * grp + qi
                        do = int(QD_OFF[q])
                        prhs = pwd[:, do:do + 4 * L]
                        bq = qi % per_bank
                        if bq == 0:
                            po = ps2.tile([128, 512], dt.float32, tag="po")
                        c0 = bq * 8 * L
                        nc.tensor.matmul(po[:, c0:c0 + 4 * L],
                                         gsb[:, qi * 256:qi * 256 + 128],
                                         prhs, start=True, stop=True)
                        nc.tensor.matmul(po[:, c0 + 4 * L:c0 + 8 * L],
                                         gsb[:, qi * 256 + 128:(qi + 1) * 256],
                                         prhs, start=True, stop=True)
                        if bq == per_bank - 1:
                            qo = int(QUAD_OFF[q - per_bank + 1]) - gbase
                            cp(ci, osb[:, qo:qo + wb], po[:, 0:wb])
                            ci += 1
                we = (nc.gpsimd, nc.sync, nc.scalar)[grp % 3]
                we.dma_start(out=outp_d[:, gbase:gbase + 32 * L], in_=osb[:])

    nc.compile()
    return nc


_CACHE = {}


def _get_compiled():
    if "nc" not in _CACHE:
        _CACHE["nc"] = _build_bass()
    return _CACHE["nc"]


def kernel(data, Pw, E_re, E_im, pad_idx):
    from concourse import bass_utils

    data = np.asarray(data)
    Pw = np.asarray(Pw, dtype=np.float32)
    E_re = np.asarray(E_re, dtype=np.float32)
    E_im = np.asarray(E_im, dtype=np.float32)

    cores, nlon = _assign_groups()
    offs = np.concatenate([[0], np.cumsum(nlon)[:-1]])
    x = np.ascontiguousarray(
        np.transpose(data, (0, 1, 3, 2)).reshape(BF, NPTS).astype(np.float32))
    PwT = np.ascontiguousarray(np.transpose(Pw, (1, 2, 0)))  # [m, n, l]

    in_maps = []
    for c in range(NCORES):
        xe, ee, pwd = _build_core(cores[c], nlon, offs, x, E_re, E_im, PwT)
        in_maps.append({"xe": xe, "ee": ee, "pwd": pwd})

    nc = _get_compiled()
    res = bass_utils.run_bass_kernel_spmd(nc, in_maps, list(range(NCORES)))
    _CACHE["last_results"] = res

    total = np.zeros((BF, OUTW), np.float64)
    for r in res.results:
        total += r["outp"].astype(np.float64)
    total = total.astype(np.float32)

    cc = np.zeros((LMAX, MMAX, BF), np.complex64)
    for q in range(32):
        grp, qi = divmod(q, 4)
        L = LLEN(16 * grp)
        lb = 16 * grp
        o = int(QUAD_OFF[q])
        for b in range(4):
            m = 16 * grp + qi + 4 * b
            re = total[:, o + b * L:o + (b + 1) * L]
            im = total[:, o + 4 * L + b * L:o + 4 * L + (b + 1) * L]
            cc[lb:, m, :] = (re + 1j * im).T
    cc = cc.reshape(LMAX, MMAX, B, V)
    out = np.transpose(cc, (2, 0, 1, 3))[:, None]
    return out.astype(np.complex64)
